# revision 7
# baseline (speedup 1.0000x reference)
"""DenseCapsule dynamic-routing kernel for 8 Trainium2 NeuronCores.

Problem: x [256,1152,8] f32, weight [10,1152,16,8] f32 ->
  x_hat = einsum('oidc,bic->boid', weight, x)
  3 rounds of routing-by-agreement (softmax over o, squash over d)
  output [256, 10, 16] f32.

Strategy (batch-parallel over 8 cores, 32 samples/core):
  - i is split as i = k*8 + g  (k in [0,144), g in [0,8)).
  - Host builds block-diagonal weight stationaries WS[o,k] of shape [64,128]:
      WS[g*8+c, g*16+d] = W[o, k*8+g, d, c]
    and moving x blocks XM[g*8+c, k, b] = x[b, k*8+g, c], both fp16.
  - PE matmul per (o,k): out[(g,d), b] = x_hat[b,o,k*8+g,d] accumulated to
    PSUM fp32, copied to SBUF as XH fp16 with layout [p=(g,d), f=(o,k,b)].
  - Routing uses linearity of the logits: b_t = x_hat . (v_0+...+v_{t-1}),
    so no logits are stored across iterations; per iteration we recompute
    them from vsum with one elementwise pass + small PE reductions:
      z   = XH * vsum_rep          (DVE)
      bp  = S2^T @ z               (PE: sums d within each g, replicated)
      e   = exp(bp)                (ACT, PSUM->SBUF)
      Z   = sum_o e ; zinv = 1/Z   (DVE)
      m   = e * XH * zinv          (DVE, in-place)
      s  += sum_k m                (DVE reduce, then PE g-sum via S1)
    squash() is computed on [128, (o,b)] tiles with g-replicated layout.
"""

import sys

for _p in ("/opt/trn_rl_repo",):
    if _p not in sys.path:
        sys.path.insert(0, _p)

import numpy as np

B, I, DIN, O, DOUT = 256, 1152, 8, 10, 16
NCORES = 8
BL = B // NCORES          # 32 samples per core
G = 8                     # i's per contraction block
NK = I // G               # 144 k blocks
NCJ = 3                   # weight DMA chunks per o
KCJ = NK // NCJ           # 48 k per DMA chunk
KPS = 16                  # k per PSUM group in phase 1
KRC = 8                   # k per routing chunk
NRC = NK // KRC           # 18 routing chunks
EPS = 1e-8

_CACHE = {}


def _build_host_constants(weight):
    """Block-diagonal stationaries + selection matrices (host side)."""
    w5 = weight.reshape(O, NK, G, DOUT, DIN)          # [o,k,g,d,c]
    ws = np.zeros((O, NK, G, DIN, G, DOUT), np.float16)
    for g in range(G):
        # ws[o,k,g,c,g,d] = w5[o,k,g,d,c]
        ws[:, :, g, :, g, :] = np.swapaxes(w5[:, :, g, :, :], -1, -2)
    # [o, cj, r=64, kc=48, m=128]
    ws = ws.reshape(O, NK, G * DIN, G * DOUT)          # [o,k,64,128]
    ws = ws.reshape(O, NCJ, KCJ, 64, 128).transpose(0, 1, 3, 2, 4).copy()

    # selection matrices, layout p=(g,d) with p = g*16+d
    gi = np.arange(128) // DOUT   # g of row
    di = np.arange(128) % DOUT    # d of row
    s1 = (di[:, None] == di[None, :]).astype(np.float32)   # g-sum, replicated
    s2 = (gi[:, None] == gi[None, :]).astype(np.float32)   # d-sum, replicated
    s3 = np.full((128, 128), 0.125, np.float32)            # full sum / 8
    return ws, s1.astype(np.float32), s2.astype(np.float16), s3


def _build_program():
    import concourse.tile as tile
    from concourse import bacc, mybir

    f16 = mybir.dt.float16
    f32 = mybir.dt.float32
    AF = mybir.ActivationFunctionType
    AX = mybir.AxisListType

    nc = bacc.Bacc(
        "TRN2",
        target_bir_lowering=False,
        debug=False,
        enable_asserts=False,
        num_devices=NCORES,
    )

    ws_d = nc.dram_tensor("ws", [O, NCJ, 64, KCJ, 128], f16, kind="ExternalInput")
    xm_d = nc.dram_tensor("xm", [64, NK, BL], f16, kind="ExternalInput")
    s1_d = nc.dram_tensor("s1", [128, 128], f32, kind="ExternalInput")
    s2_d = nc.dram_tensor("s2", [128, 128], f16, kind="ExternalInput")
    s3_d = nc.dram_tensor("s3", [128, 128], f32, kind="ExternalInput")
    out_d = nc.dram_tensor("out", [DOUT, O, BL], f32, kind="ExternalOutput")

    with tile.TileContext(nc) as tc:
        with (
            tc.tile_pool(name="const", bufs=1) as const,
            tc.tile_pool(name="wpool", bufs=2) as wpool,
            tc.tile_pool(name="xhp", bufs=1) as xhp,
            tc.tile_pool(name="small", bufs=2) as small,
            tc.tile_pool(name="acc", bufs=1) as acc,
            tc.tile_pool(name="zp", bufs=2) as zp,
            tc.tile_pool(name="ep", bufs=2) as ep,
            tc.tile_pool(name="spsum", bufs=1, space="PSUM") as spsum,
        ):
            xm_sb = const.tile([64, NK, BL], f16)
            nc.gpsimd.dma_start(out=xm_sb[:], in_=xm_d.ap())
            s1_sb = const.tile([128, 128], f32)
            nc.gpsimd.dma_start(out=s1_sb[:], in_=s1_d.ap())
            s2_sb = const.tile([128, 128], f16)
            nc.gpsimd.dma_start(out=s2_sb[:], in_=s2_d.ap())
            s3_sb = const.tile([128, 128], f32)
            nc.gpsimd.dma_start(out=s3_sb[:], in_=s3_d.ap())

            xh = xhp.tile([128, O, NK, BL], f16)       # x_hat, p=(g,d)
            s0p = acc.tile([128, O, BL], f32)          # t=0 per-o k-sums

            # ---- Phase 1: x_hat = W @ x ------------------------------------
            with tc.tile_pool(name="ppsum", bufs=4, space="PSUM") as ppsum:
                for o in range(O):
                    for cj in range(NCJ):
                        wck = wpool.tile([64, KCJ, 128], f16)
                        nc.gpsimd.dma_start(out=wck[:], in_=ws_d.ap()[o, cj])
                        for pj in range(KCJ // KPS):
                            pt = ppsum.tile([128, KPS, BL], f32)
                            for kk in range(KPS):
                                k = cj * KCJ + pj * KPS + kk
                                nc.tensor.matmul(
                                    pt[:, kk, :],
                                    lhsT=wck[:, pj * KPS + kk, :],
                                    rhs=xm_sb[:, k, :],
                                    start=True,
                                    stop=True,
                                )
                            nc.scalar.copy(
                                out=xh[:, o, cj * KCJ + pj * KPS:
                                       cj * KCJ + (pj + 1) * KPS, :],
                                in_=pt[:],
                            )
                    # t=0 partial: sum over k (uniform routing weights)
                    nc.vector.reduce_sum(
                        out=s0p[:, o, :],
                        in_=xh[:, o, :, :].transpose([0, 2, 1]),
                        axis=AX.X,
                    )

            def squash(s_psum_ap, scale):
                """s_psum [128,(o,b)] fp32 g-replicated sums -> v [128,(o,b)] f32."""
                s_sb = small.tile([128, O, BL], f32, tag="sq_s")
                nc.scalar.mul(out=s_sb[:], in_=s_psum_ap, mul=scale)
                sq = small.tile([128, O, BL], f32, tag="sq_sq")
                nc.vector.tensor_mul(sq[:], s_sb[:], s_sb[:])
                m2p = spsum.tile([128, O, BL], f32, tag="sq_m2")
                nc.tensor.matmul(m2p[:], lhsT=s3_sb[:], rhs=sq[:],
                                 start=True, stop=True)
                rt = small.tile([128, O, BL], f32, tag="sq_rt")
                nc.scalar.sqrt(out=rt[:], in_=m2p[:])       # sqrt(mag2)
                nc.vector.tensor_scalar_add(rt[:], rt[:], EPS)
                den = small.tile([128, O, BL], f32, tag="sq_den")
                nc.scalar.add(out=den[:], in_=m2p[:], add=1.0)  # 1+mag2
                nc.vector.tensor_mul(den[:], den[:], rt[:])
                nc.vector.reciprocal(out=den[:], in_=den[:])
                fac = small.tile([128, O, BL], f32, tag="sq_fac")
                nc.vector.tensor_mul(fac[:], m2p[:], den[:])
                v = small.tile([128, O, BL], f32, tag="sq_v")
                nc.vector.tensor_mul(v[:], s_sb[:], fac[:])
                return v

            # ---- t = 0: uniform c = 1/10 -----------------------------------
            srp = spsum.tile([128, O, BL], f32, tag="srp")
            nc.tensor.matmul(srp[:], lhsT=s1_sb[:], rhs=s0p[:],
                             start=True, stop=True)
            v = squash(srp[:], 1.0 / O)
            vsum = acc.tile([128, O, BL], f32)
            nc.vector.tensor_copy(out=vsum[:], in_=v[:])
            vsum16 = acc.tile([128, O, BL], f16)
            nc.scalar.copy(out=vsum16[:], in_=vsum[:])

            # ---- t = 1, 2 ---------------------------------------------------
            sparts = acc.tile([128, NRC, O, BL], f32)
            with tc.tile_pool(name="bpsum", bufs=1, space="PSUM") as bpsum:
                for t in (1, 2):
                    for kc in range(NRC):
                        ks = kc * KRC
                        # z = XH * vsum (broadcast over k)
                        z = zp.tile([128, O, KRC, BL], f16)
                        nc.vector.tensor_mul(
                            z[:],
                            xh[:, :, ks:ks + KRC, :],
                            vsum16[:].unsqueeze(2).broadcast_to(
                                (128, O, KRC, BL)),
                        )
                        # logits (g-grouped d-sums), replicated over d slots
                        bp = bpsum.tile([128, O, KRC, BL], f32)
                        for o in range(O):
                            nc.tensor.matmul(bp[:, o], lhsT=s2_sb[:],
                                             rhs=z[:, o], start=True, stop=True)
                        # e = exp(logits)
                        e = ep.tile([128, O, KRC, BL], f16)
                        nc.scalar.activation(out=e[:], in_=bp[:], func=AF.Exp)
                        # Z = sum_o e ; zinv = 1/Z
                        zden = small.tile([128, KRC, BL], f32, tag="zden")
                        nc.vector.reduce_sum(
                            out=zden[:], in_=e[:].transpose([0, 2, 3, 1]),
                            axis=AX.X)
                        nc.vector.reciprocal(out=zden[:], in_=zden[:])
                        zinv16 = small.tile([128, KRC, BL], f16, tag="zinv")
                        nc.scalar.copy(out=zinv16[:], in_=zden[:])
                        # m = e * XH * zinv  (in place on e)
                        nc.vector.tensor_mul(e[:], e[:],
                                             xh[:, :, ks:ks + KRC, :])
                        nc.vector.tensor_mul(
                            e[:], e[:],
                            zinv16[:].unsqueeze(1).broadcast_to(
                                (128, O, KRC, BL)),
                        )
                        # s partial: sum over k in chunk
                        nc.vector.reduce_sum(
                            out=sparts[:, kc], in_=e[:].transpose([0, 1, 3, 2]),
                            axis=AX.X)
                    stot = small.tile([128, O, BL], f32, tag="stot")
                    nc.vector.reduce_sum(
                        out=stot[:], in_=sparts[:].transpose([0, 2, 3, 1]),
                        axis=AX.X)
                    srp2 = spsum.tile([128, O, BL], f32, tag="srp")
                    nc.tensor.matmul(srp2[:], lhsT=s1_sb[:], rhs=stot[:],
                                     start=True, stop=True)
                    v = squash(srp2[:], 1.0)
                    if t == 1:
                        nc.vector.tensor_add(vsum[:], vsum[:], v[:])
                        nc.scalar.copy(out=vsum16[:], in_=vsum[:])
                    else:
                        nc.gpsimd.dma_start(out=out_d.ap(), in_=v[0:DOUT])

    nc.compile()
    return nc


def _prepare_in_maps(inputs):
    x = np.asarray(inputs["x"], np.float32)
    weight = np.asarray(inputs["weight"], np.float32)
    ws, s1, s2, s3 = _build_host_constants(weight)

    # moving x blocks: xm[g*8+c, k, b] = x[b, k*8+g, c]
    x6 = x.reshape(B, NK, G, DIN)
    in_maps = []
    for core in range(NCORES):
        xl = x6[core * BL:(core + 1) * BL]                 # [b,k,g,c]
        xm = xl.transpose(2, 3, 1, 0).reshape(64, NK, BL)  # [(g,c),k,b]
        in_maps.append({
            "ws": ws,
            "xm": np.ascontiguousarray(xm, np.float16),
            "s1": s1,
            "s2": s2,
            "s3": s3,
        })
    return in_maps


def kernel(x, weight):
    from concourse.bass_utils import run_bass_kernel_spmd

    if "nc" not in _CACHE:
        _CACHE["nc"] = _build_program()
    nc = _CACHE["nc"]

    in_maps = _prepare_in_maps({"x": x, "weight": weight})

    res = run_bass_kernel_spmd(nc, in_maps, core_ids=list(range(NCORES)))
    _CACHE["last_results"] = res

    out = np.empty((B, O, DOUT), np.float32)
    for core in range(NCORES):
        oc = res.results[core]["out"]                      # [d, o, b]
        out[core * BL:(core + 1) * BL] = oc.transpose(2, 1, 0)
    return out


# revision 9
# speedup vs baseline: 97.0386x; 97.0386x over previous
"""DenseCapsule dynamic-routing kernel for 8 Trainium2 NeuronCores.

Problem: x [256,1152,8] f32, weight [10,1152,16,8] f32 ->
  x_hat = einsum('oidc,bic->boid', weight, x)
  3 rounds of routing-by-agreement (softmax over o, squash over d)
  output [256, 10, 16] f32.

Strategy (batch-parallel over 8 cores, 32 samples/core):
  - i is split as i = k*8 + g  (k in [0,144), g in [0,8)).
  - Host builds block-diagonal weight stationaries WS[o,k] of shape [64,128]:
      WS[g*8+c, g*16+d] = W[o, k*8+g, d, c]
    and moving x blocks XM[g*8+c, k, b] = x[b, k*8+g, c], both fp16.
  - PE matmul per (o,k): out[(g,d), b] = x_hat[b,o,k*8+g,d] accumulated to
    PSUM fp32, copied to SBUF as XH fp16 with layout [p=(g,d), f=(o,k,b)].
  - Routing uses linearity of the logits: b_t = x_hat . (v_0+...+v_{t-1}),
    so no logits are stored across iterations; per iteration we recompute
    them from vsum with one elementwise pass + small PE reductions:
      z   = XH * vsum_rep          (DVE)
      bp  = S2^T @ z               (PE: sums d within each g, replicated)
      e   = exp(bp)                (ACT, PSUM->SBUF)
      Z   = sum_o e ; zinv = 1/Z   (DVE)
      m   = e * XH * zinv          (DVE, in-place)
      s  += sum_k m                (DVE reduce, then PE g-sum via S1)
    squash() is computed on [128, (o,b)] tiles with g-replicated layout.
"""

import sys

for _p in ("/opt/trn_rl_repo",):
    if _p not in sys.path:
        sys.path.insert(0, _p)

import numpy as np

B, I, DIN, O, DOUT = 256, 1152, 8, 10, 16
NCORES = 8
BL = B // NCORES          # 32 samples per core
G = 8                     # i's per contraction block
NK = I // G               # 144 k blocks
NCJ = 3                   # weight DMA chunks per o
KCJ = NK // NCJ           # 48 k per DMA chunk
KPS = 16                  # k per PSUM group in phase 1
KRC = 8                   # k per routing chunk
NRC = NK // KRC           # 18 routing chunks
EPS = 1e-8

_CACHE = {}


def _build_host_constants(weight):
    """Block-diagonal stationaries + selection matrices (host side)."""
    w5 = weight.reshape(O, NK, G, DOUT, DIN)          # [o,k,g,d,c]
    ws = np.zeros((O, NK, G, DIN, G, DOUT), np.float16)
    for g in range(G):
        # ws[o,k,g,c,g,d] = w5[o,k,g,d,c]
        ws[:, :, g, :, g, :] = np.swapaxes(w5[:, :, g, :, :], -1, -2)
    # [o, cj, r=64, kc=48, m=128]
    ws = ws.reshape(O, NK, G * DIN, G * DOUT)          # [o,k,64,128]
    ws = ws.reshape(O, NCJ, KCJ, 64, 128).transpose(0, 1, 3, 2, 4).copy()

    # selection matrices, layout p=(g,d) with p = g*16+d
    gi = np.arange(128) // DOUT   # g of row
    di = np.arange(128) % DOUT    # d of row
    s1 = (di[:, None] == di[None, :]).astype(np.float32)   # g-sum, replicated
    s2 = (gi[:, None] == gi[None, :]).astype(np.float32)   # d-sum, replicated
    s3 = np.full((128, 128), 0.125, np.float32)            # full sum / 8
    return ws, s1.astype(np.float32), s2.astype(np.float16), s3


def _build_program(routing_iters=2, phase1=True):
    import concourse.tile as tile
    from concourse import bacc, mybir

    f16 = mybir.dt.float16
    f32 = mybir.dt.float32
    AF = mybir.ActivationFunctionType
    AX = mybir.AxisListType

    nc = bacc.Bacc(
        "TRN2",
        target_bir_lowering=False,
        debug=False,
        enable_asserts=False,
        num_devices=NCORES,
    )

    ws_d = nc.dram_tensor("ws", [O, NCJ, 64, KCJ, 128], f16, kind="ExternalInput")
    xm_d = nc.dram_tensor("xm", [64, NK, BL], f16, kind="ExternalInput")
    s1_d = nc.dram_tensor("s1", [128, 128], f32, kind="ExternalInput")
    s2_d = nc.dram_tensor("s2", [128, 128], f16, kind="ExternalInput")
    s3_d = nc.dram_tensor("s3", [128, 128], f32, kind="ExternalInput")
    out_d = nc.dram_tensor("out", [DOUT, O, BL], f32, kind="ExternalOutput")

    with tile.TileContext(nc) as tc:
        with (
            tc.tile_pool(name="const", bufs=1) as const,
            tc.tile_pool(name="wpool", bufs=2) as wpool,
            tc.tile_pool(name="xhp", bufs=1) as xhp,
            tc.tile_pool(name="small", bufs=2) as small,
            tc.tile_pool(name="acc", bufs=1) as acc,
            tc.tile_pool(name="zp", bufs=2) as zp,
            tc.tile_pool(name="ep", bufs=2) as ep,
            tc.tile_pool(name="spsum", bufs=1, space="PSUM") as spsum,
        ):
            xm_sb = const.tile([64, NK, BL], f16)
            nc.gpsimd.dma_start(out=xm_sb[:], in_=xm_d.ap())
            s1_sb = const.tile([128, 128], f32)
            nc.gpsimd.dma_start(out=s1_sb[:], in_=s1_d.ap())
            s2_sb = const.tile([128, 128], f16)
            nc.gpsimd.dma_start(out=s2_sb[:], in_=s2_d.ap())
            s3_sb = const.tile([128, 128], f32)
            nc.gpsimd.dma_start(out=s3_sb[:], in_=s3_d.ap())

            xh = xhp.tile([128, O, NK, BL], f16)       # x_hat, p=(g,d)
            s0p = acc.tile([128, O, BL], f32)          # t=0 per-o k-sums

            # ---- Phase 1: x_hat = W @ x ------------------------------------
            with tc.tile_pool(name="ppsum", bufs=4, space="PSUM") as ppsum:
                for o in range(O):
                    for cj in range(NCJ if phase1 else 0):
                        wck = wpool.tile([64, KCJ, 128], f16)
                        nc.gpsimd.dma_start(out=wck[:], in_=ws_d.ap()[o, cj])
                        for pj in range(KCJ // KPS):
                            pt = ppsum.tile([128, KPS, BL], f32)
                            for kk in range(KPS):
                                k = cj * KCJ + pj * KPS + kk
                                nc.tensor.matmul(
                                    pt[:, kk, :],
                                    lhsT=wck[:, pj * KPS + kk, :],
                                    rhs=xm_sb[:, k, :],
                                    start=True,
                                    stop=True,
                                )
                            nc.scalar.copy(
                                out=xh[:, o, cj * KCJ + pj * KPS:
                                       cj * KCJ + (pj + 1) * KPS, :],
                                in_=pt[:],
                            )
                    # t=0 partial: sum over k (uniform routing weights)
                    nc.vector.reduce_sum(
                        out=s0p[:, o, :],
                        in_=xh[:, o, :, :].transpose([0, 2, 1]),
                        axis=AX.X,
                    )

            def squash(s_psum_ap, scale):
                """s_psum [128,(o,b)] fp32 g-replicated sums -> v [128,(o,b)] f32."""
                s_sb = small.tile([128, O, BL], f32, tag="sq_s")
                nc.scalar.mul(out=s_sb[:], in_=s_psum_ap, mul=scale)
                sq = small.tile([128, O, BL], f32, tag="sq_sq")
                nc.vector.tensor_mul(sq[:], s_sb[:], s_sb[:])
                m2p = spsum.tile([128, O, BL], f32, tag="sq_m2")
                nc.tensor.matmul(m2p[:], lhsT=s3_sb[:], rhs=sq[:],
                                 start=True, stop=True)
                rt = small.tile([128, O, BL], f32, tag="sq_rt")
                nc.scalar.sqrt(out=rt[:], in_=m2p[:])       # sqrt(mag2)
                nc.vector.tensor_scalar_add(rt[:], rt[:], EPS)
                den = small.tile([128, O, BL], f32, tag="sq_den")
                nc.scalar.add(out=den[:], in_=m2p[:], add=1.0)  # 1+mag2
                nc.vector.tensor_mul(den[:], den[:], rt[:])
                nc.vector.reciprocal(out=den[:], in_=den[:])
                fac = small.tile([128, O, BL], f32, tag="sq_fac")
                nc.vector.tensor_mul(fac[:], m2p[:], den[:])
                v = small.tile([128, O, BL], f32, tag="sq_v")
                nc.vector.tensor_mul(v[:], s_sb[:], fac[:])
                return v

            # ---- t = 0: uniform c = 1/10 -----------------------------------
            srp = spsum.tile([128, O, BL], f32, tag="srp")
            nc.tensor.matmul(srp[:], lhsT=s1_sb[:], rhs=s0p[:],
                             start=True, stop=True)
            v = squash(srp[:], 1.0 / O)
            vsum = acc.tile([128, O, BL], f32)
            nc.vector.tensor_copy(out=vsum[:], in_=v[:])
            vsum16 = acc.tile([128, O, BL], f16)
            nc.scalar.copy(out=vsum16[:], in_=vsum[:])

            # ---- t = 1, 2 ---------------------------------------------------
            sparts = acc.tile([128, NRC, O, BL], f32)
            with tc.tile_pool(name="bpsum", bufs=1, space="PSUM") as bpsum:
                for t in (1, 2):
                    for kc in range(NRC):
                        ks = kc * KRC
                        # z = XH * vsum (broadcast over k)
                        z = zp.tile([128, O, KRC, BL], f16)
                        nc.vector.tensor_mul(
                            z[:],
                            xh[:, :, ks:ks + KRC, :],
                            vsum16[:].unsqueeze(2).broadcast_to(
                                (128, O, KRC, BL)),
                        )
                        # logits (g-grouped d-sums), replicated over d slots
                        bp = bpsum.tile([128, O, KRC, BL], f32)
                        for o in range(O):
                            nc.tensor.matmul(bp[:, o], lhsT=s2_sb[:],
                                             rhs=z[:, o], start=True, stop=True)
                        # e = exp(logits)
                        e = ep.tile([128, O, KRC, BL], f16)
                        nc.scalar.activation(out=e[:], in_=bp[:], func=AF.Exp)
                        # Z = sum_o e ; zinv = 1/Z
                        zden = small.tile([128, KRC, BL], f32, tag="zden")
                        nc.vector.reduce_sum(
                            out=zden[:], in_=e[:].transpose([0, 2, 3, 1]),
                            axis=AX.X)
                        nc.vector.reciprocal(out=zden[:], in_=zden[:])
                        zinv16 = small.tile([128, KRC, BL], f16, tag="zinv")
                        nc.scalar.copy(out=zinv16[:], in_=zden[:])
                        # m = e * XH * zinv  (in place on e)
                        nc.vector.tensor_mul(e[:], e[:],
                                             xh[:, :, ks:ks + KRC, :])
                        nc.vector.tensor_mul(
                            e[:], e[:],
                            zinv16[:].unsqueeze(1).broadcast_to(
                                (128, O, KRC, BL)),
                        )
                        # s partial: sum over k in chunk
                        nc.vector.reduce_sum(
                            out=sparts[:, kc], in_=e[:].transpose([0, 1, 3, 2]),
                            axis=AX.X)
                    stot = small.tile([128, O, BL], f32, tag="stot")
                    nc.vector.reduce_sum(
                        out=stot[:], in_=sparts[:].transpose([0, 2, 3, 1]),
                        axis=AX.X)
                    srp2 = spsum.tile([128, O, BL], f32, tag="srp")
                    nc.tensor.matmul(srp2[:], lhsT=s1_sb[:], rhs=stot[:],
                                     start=True, stop=True)
                    v = squash(srp2[:], 1.0)
                    if t == 1:
                        nc.vector.tensor_add(vsum[:], vsum[:], v[:])
                        nc.scalar.copy(out=vsum16[:], in_=vsum[:])
                    else:
                        nc.gpsimd.dma_start(out=out_d.ap(), in_=v[0:DOUT])

    nc.compile()
    return nc


def _prepare_in_maps(inputs):
    x = np.asarray(inputs["x"], np.float32)
    weight = np.asarray(inputs["weight"], np.float32)
    ws, s1, s2, s3 = _build_host_constants(weight)

    # moving x blocks: xm[g*8+c, k, b] = x[b, k*8+g, c]
    x6 = x.reshape(B, NK, G, DIN)
    in_maps = []
    for core in range(NCORES):
        xl = x6[core * BL:(core + 1) * BL]                 # [b,k,g,c]
        xm = xl.transpose(2, 3, 1, 0).reshape(64, NK, BL)  # [(g,c),k,b]
        in_maps.append({
            "ws": ws,
            "xm": np.ascontiguousarray(xm, np.float16),
            "s1": s1,
            "s2": s2,
            "s3": s3,
        })
    return in_maps


def kernel(x, weight):
    from concourse.bass_utils import run_bass_kernel_spmd

    if "nc" not in _CACHE:
        _CACHE["nc"] = _build_program()
    nc = _CACHE["nc"]

    in_maps = _prepare_in_maps({"x": x, "weight": weight})

    res = run_bass_kernel_spmd(nc, in_maps, core_ids=list(range(NCORES)))
    _CACHE["last_results"] = res

    out = np.empty((B, O, DOUT), np.float32)
    for core in range(NCORES):
        oc = res.results[core]["out"]                      # [d, o, b]
        out[core * BL:(core + 1) * BL] = oc.transpose(2, 1, 0)
    return out
